# revision 6
# baseline (speedup 1.0000x reference)
"""Trainium2 Bass kernel for ContextualLoss.

Reference computation (per batch b):
  xn = x / max(||x||_C, 1e-12)  (channel-wise L2 normalize, C=64)
  yn likewise
  dist[i,j] = || xn[:,i] - yn[:,j] ||  over N=HW=4096 spatial positions
  d_min[i]  = min_j dist[i,j]
  w[i,j]    = exp((1 - dist[i,j]/(d_min[i]+1e-5)) / 0.1)
  cx_max[i] = max_j w[i,j] / sum_j w[i,j]
  loss      = -log(mean(cx_max + 1e-5))

Sharding: 8 cores = 4 batches x 2 row-halves. Each core computes a
[2048 x 4096] block of the distance matrix, fully fused on-chip:
  G = x_raw^T @ yn  (PE, K=C=64)
  dist = Sqrt(-2*rx*G + 2)      (ACT, per-partition scale rx = 1/||x_i||)
  row max of G -> d_min          (DVE reduce)
  w = Exp(-10/(d_min+eps)*dist + 10), row-sum via activation accumulate
  cx_max = w_max / w_sum  -> [128, 16] per core, final -log(mean) on host.
"""

import os
import sys

import numpy as np

sys.path.insert(0, "/opt/trn_rl_repo")

B = 4
C = 64
N = 4096          # H*W
NCORES = 8
ROWS = 2048       # rows of the distance matrix per core (N*B/NCORES)
NBLK = ROWS // 128  # 16 row blocks
YCHUNKS = N // 128  # 32
INV_SIGMA = 10.0    # 1/SIGMA with SIGMA=0.1; B_PARAM=1 -> bias = 10.0
EPS_MIN = 1e-5
EPS_NORM = 1e-12

_CACHE = {}


def _build_nc():
    import concourse.bass as bass
    import concourse.mybir as mybir
    from concourse import bacc, tile
    from concourse.masks import make_identity
    from contextlib import ExitStack

    f32 = mybir.dt.float32
    AF = mybir.ActivationFunctionType
    OP = mybir.AluOpType
    AX = mybir.AxisListType

    nc = bacc.Bacc("TRN2", target_bir_lowering=False, debug=False)

    xw_d = nc.dram_tensor("xw", [C, ROWS], f32, kind="ExternalInput").ap()
    xt_d = nc.dram_tensor("xt", [ROWS, C], f32, kind="ExternalInput").ap()
    yt_d = nc.dram_tensor("yt", [N, C], f32, kind="ExternalInput").ap()
    out_d = nc.dram_tensor("out", [128, NBLK], f32, kind="ExternalOutput").ap()

    with ExitStack() as ctx:
        tc = ctx.enter_context(tile.TileContext(nc))
        const = ctx.enter_context(tc.tile_pool(name="const", bufs=1))
        work = ctx.enter_context(tc.tile_pool(name="work", bufs=3))
        psum = ctx.enter_context(tc.tile_pool(name="psum", bufs=2, space="PSUM"))

        # ---------------- persistent tiles ----------------
        xw = const.tile([C, ROWS], f32)           # raw x slice, matmul weights
        xt = const.tile([128, NBLK, C], f32)      # x transposed: [p, blk, c]
        yt = const.tile([128, YCHUNKS, C], f32)   # y transposed: [p, chunk, c]
        yn = const.tile([C, N], f32)              # normalized y, matmul rhs
        ynt = const.tile([128, YCHUNKS, C], f32)  # normalized y, transposed
        ident = const.tile([128, 128], f32)
        b2 = const.tile([128, 1], f32)
        b10 = const.tile([128, 1], f32)

        # per-row statistics, [128, NBLK]-shaped
        s2x = const.tile([128, NBLK], f32)
        rx = const.tile([128, NBLK], f32)
        neg2rx = const.tile([128, NBLK], f32)
        s2y = const.tile([128, YCHUNKS], f32)
        ry = const.tile([128, YCHUNKS], f32)
        gmx = const.tile([128, 2 * NBLK], f32)    # per-half row max of G
        gmax = const.tile([128, NBLK], f32)
        dmin = const.tile([128, NBLK], f32)
        dme = const.tile([128, NBLK], f32)        # dmin + eps
        arec = const.tile([128, NBLK], f32)       # 1/(dmin+eps)
        sc2 = const.tile([128, NBLK], f32)        # -10/(dmin+eps)
        wsh = const.tile([128, 2 * NBLK], f32)    # per-half w row sums
        wsum = const.tile([128, NBLK], f32)
        tdm = const.tile([128, NBLK], f32)        # dmin * arec
        wmax = const.tile([128, NBLK], f32)
        rsum = const.tile([128, NBLK], f32)
        cx = const.tile([128, NBLK], f32)

        # ---------------- load inputs ----------------
        nc.sync.dma_start(out=xw, in_=xw_d)
        nc.sync.dma_start(out=xt, in_=xt_d.rearrange("(k p) c -> p k c", p=128))
        nc.sync.dma_start(out=yt, in_=yt_d.rearrange("(k p) c -> p k c", p=128))

        make_identity(nc, ident)
        nc.vector.memset(b2, 2.0)
        nc.vector.memset(b10, 10.0)

        # ---------------- norms (compact layouts) ----------------
        sqx = work.tile([128, NBLK * C], f32, tag="sq")
        nc.vector.tensor_mul(sqx, xt[:, :, :].rearrange("p k c -> p (k c)"),
                             xt[:, :, :].rearrange("p k c -> p (k c)"))
        nc.vector.reduce_sum(s2x, sqx[:].rearrange("p (k c) -> p k c", c=C),
                             axis=AX.X)
        sqy = work.tile([128, YCHUNKS * C], f32, tag="sq")
        nc.vector.tensor_mul(sqy, yt[:, :, :].rearrange("p k c -> p (k c)"),
                             yt[:, :, :].rearrange("p k c -> p (k c)"))
        nc.vector.reduce_sum(s2y, sqy[:].rearrange("p (k c) -> p k c", c=C),
                             axis=AX.X)

        # rx = 1/max(sqrt(s2x), eps); neg2rx = -2*rx
        nrmx = work.tile([128, NBLK], f32, tag="nrm")
        nc.scalar.activation(nrmx, s2x, AF.Sqrt)
        nc.vector.tensor_scalar_max(nrmx, nrmx, EPS_NORM)
        nc.vector.reciprocal(rx, nrmx)
        nc.vector.tensor_scalar_mul(neg2rx, rx, -2.0)

        nrmy = work.tile([128, YCHUNKS], f32, tag="nrm")
        nc.scalar.activation(nrmy, s2y, AF.Sqrt)
        nc.vector.tensor_scalar_max(nrmy, nrmy, EPS_NORM)
        nc.vector.reciprocal(ry, nrmy)

        # ---------------- build yn = y * ry  (via transposed layout) -------
        for k in range(YCHUNKS):
            nc.vector.tensor_scalar_mul(ynt[:, k, :], yt[:, k, :], ry[:, k : k + 1])
        for h in range(2):
            pyn = psum.tile([64, 2048], f32, tag="G")
            for k in range(16):
                kk = 16 * h + k
                nc.tensor.transpose(
                    out=pyn[0:64, 128 * k : 128 * (k + 1)],
                    in_=ynt[:, kk, :],
                    identity=ident,
                )
            # copy PSUM -> SBUF (split across engines)
            if h == 0:
                nc.vector.tensor_copy(yn[0:64, 0:2048], pyn)
            else:
                nc.scalar.copy(yn[0:64, 2048:4096], pyn)

        # Collapse all preproc deps into one barrier: without it the first
        # main-loop LDWEIGHTS accumulates more sync waits than the ISA allows.
        tc.strict_bb_all_engine_barrier()

        # ---------------- main loop ----------------
        for i in range(NBLK):
            lhsT = xw[:, 128 * i : 128 * (i + 1)]
            dists = []
            for h in range(2):
                g = psum.tile([128, 2048], f32, tag="G")
                for n in range(4):
                    col = 2048 * h + 512 * n
                    nc.tensor.matmul(
                        g[:, 512 * n : 512 * (n + 1)],
                        lhsT,
                        yn[0:64, col : col + 512],
                        start=True,
                        stop=True,
                    )
                ih = 2 * i + h
                # row max of G (-> min distance)
                nc.vector.reduce_max(gmx[:, ih : ih + 1], g, axis=AX.X)
                # dist = sqrt(2 - 2*rx*G)
                dist = work.tile([128, 2048], f32, tag="dist")
                nc.scalar.activation(
                    dist, g, AF.Sqrt, bias=b2, scale=neg2rx[:, i : i + 1]
                )
                dists.append(dist)
            # per-block scalar chain -> sc2[:, i] (the Exp scale below)
            nc.vector.tensor_max(
                gmax[:, i : i + 1], gmx[:, 2 * i : 2 * i + 1], gmx[:, 2 * i + 1 : 2 * i + 2]
            )
            nc.scalar.activation(
                dmin[:, i : i + 1], gmax[:, i : i + 1], AF.Sqrt,
                bias=b2, scale=neg2rx[:, i : i + 1],
            )
            nc.vector.tensor_scalar_add(dme[:, i : i + 1], dmin[:, i : i + 1], EPS_MIN)
            nc.vector.reciprocal(arec[:, i : i + 1], dme[:, i : i + 1])
            nc.vector.tensor_scalar_mul(sc2[:, i : i + 1], arec[:, i : i + 1], -INV_SIGMA)
            for h in range(2):
                ih = 2 * i + h
                # w = exp(10 - 10*dist/(dmin+eps)); accumulate row sum
                wscr = work.tile([128, 2048], f32, tag="wscr")
                nc.scalar.activation(
                    wscr,
                    dists[h],
                    AF.Exp,
                    bias=b10,
                    scale=sc2[:, i : i + 1],
                    accum_out=wsh[:, ih : ih + 1],
                )

        # ---------------- epilogue ----------------
        nc.vector.reduce_sum(
            wsum, wsh[:].rearrange("p (i h) -> p i h", h=2), axis=AX.X
        )
        nc.vector.tensor_mul(tdm, dmin, arec)
        nc.scalar.activation(wmax, tdm, AF.Exp, bias=b10, scale=-INV_SIGMA)
        nc.vector.reciprocal(rsum, wsum)
        nc.vector.tensor_mul(cx, wmax, rsum)
        nc.sync.dma_start(out=out_d, in_=cx)

    return nc


def _get_nc(finalized=True):
    """Build (once) and return the Bass module.

    run_bass_via_pjrt serializes the module without calling finalize(), but
    Bacc's legalization passes (register allocation, matmul wait splitting,
    event semaphores) only run in finalize() — so finalize here.
    """
    if "nc" not in _CACHE:
        nc = _build_nc()
        if finalized:
            nc.finalize()
        _CACHE["nc"] = nc
    return _CACHE["nc"]


def _make_in_maps(x, y):
    x = np.ascontiguousarray(np.asarray(x, dtype=np.float32)).reshape(B, C, N)
    y = np.ascontiguousarray(np.asarray(y, dtype=np.float32)).reshape(B, C, N)
    in_maps = []
    for c in range(NCORES):
        b, h = divmod(c, 2)
        xs = x[b][:, 2048 * h : 2048 * (h + 1)]
        in_maps.append(
            {
                "xw": np.ascontiguousarray(xs),
                "xt": np.ascontiguousarray(xs.T),
                "yt": np.ascontiguousarray(y[b].T),
            }
        )
    return in_maps


def _finish(outs):
    """outs: list of 8 arrays [128, NBLK] -> scalar loss (float32 0-d)."""
    total = 0.0
    for o in outs:
        total += float(np.asarray(o, dtype=np.float64).sum())
    mean = total / (B * N) + EPS_MIN
    return np.array(-np.log(mean), dtype=np.float32)


def kernel(x, y):
    from concourse.bass_utils import run_bass_kernel_spmd

    nc = _get_nc()
    in_maps = _make_in_maps(x, y)
    res = run_bass_kernel_spmd(nc, in_maps, core_ids=list(range(NCORES)))
    outs = [res.results[c]["out"] for c in range(NCORES)]
    return _finish(outs)


# revision 9
# speedup vs baseline: 1.2123x; 1.2123x over previous
"""Trainium2 Bass kernel for ContextualLoss.

Reference computation (per batch b):
  xn = x / max(||x||_C, 1e-12)  (channel-wise L2 normalize, C=64)
  yn likewise
  dist[i,j] = || xn[:,i] - yn[:,j] ||  over N=HW=4096 spatial positions
  d_min[i]  = min_j dist[i,j]
  w[i,j]    = exp((1 - dist[i,j]/(d_min[i]+1e-5)) / 0.1)
  cx_max[i] = max_j w[i,j] / sum_j w[i,j]
  loss      = -log(mean(cx_max + 1e-5))

Sharding: 8 cores = 4 batches x 2 row-halves. Each core computes a
[2048 x 4096] block of the distance matrix, fully fused on-chip:
  G = x_raw^T @ yn  (PE, K=C=64)
  dist = Sqrt(-2*rx*G + 2)      (ACT, per-partition scale rx = 1/||x_i||)
  row max of G -> d_min          (DVE reduce)
  w = Exp(-10/(d_min+eps)*dist + 10), row-sum via activation accumulate
  cx_max = w_max / w_sum  -> [128, 16] per core, final -log(mean) on host.
"""

import os
import sys

import numpy as np

sys.path.insert(0, "/opt/trn_rl_repo")

B = 4
C = 64
N = 4096          # H*W
NCORES = 8
ROWS = 2048       # rows of the distance matrix per core (N*B/NCORES)
NBLK = ROWS // 128  # 16 row blocks
YCHUNKS = N // 128  # 32
INV_SIGMA = 10.0    # 1/SIGMA with SIGMA=0.1; B_PARAM=1 -> bias = 10.0
EPS_MIN = 1e-5
EPS_NORM = 1e-12

_CACHE = {}


def _build_nc():
    import concourse.bass as bass
    import concourse.mybir as mybir
    from concourse import bacc, tile
    from concourse.tile import add_dep_helper
    from concourse.masks import make_identity
    from contextlib import ExitStack

    f32 = mybir.dt.float32
    f32r = mybir.dt.float32r
    AF = mybir.ActivationFunctionType
    OP = mybir.AluOpType
    AX = mybir.AxisListType

    nc = bacc.Bacc("TRN2", target_bir_lowering=False, debug=False)

    xw_d = nc.dram_tensor("xw", [C, ROWS], f32, kind="ExternalInput").ap()
    xt_d = nc.dram_tensor("xt", [ROWS, C], f32, kind="ExternalInput").ap()
    yt_d = nc.dram_tensor("yt", [N, C], f32, kind="ExternalInput").ap()
    out_d = nc.dram_tensor("out", [128, NBLK], f32, kind="ExternalOutput").ap()

    with ExitStack() as ctx:
        tc = ctx.enter_context(tile.TileContext(nc))
        const = ctx.enter_context(tc.tile_pool(name="const", bufs=1))
        work = ctx.enter_context(tc.tile_pool(name="work", bufs=3))
        psum = ctx.enter_context(tc.tile_pool(name="psum", bufs=2, space="PSUM"))
        distp = ctx.enter_context(tc.tile_pool(name="distp", bufs=9))

        # ---------------- persistent tiles ----------------
        xw = const.tile([C, ROWS], f32)           # raw x slice (DMA target)
        xwr = const.tile([C, ROWS], f32r)         # f32r-rounded matmul weights
        xt = const.tile([128, NBLK, C], f32)      # x transposed: [p, blk, c]
        yt = const.tile([128, YCHUNKS, C], f32)   # y transposed: [p, chunk, c]
        yn = const.tile([C, N], f32r)             # normalized y, matmul rhs
        ynt = const.tile([128, YCHUNKS, C], f32)  # normalized y, transposed
        ident = const.tile([128, 128], f32)
        b2 = const.tile([128, 1], f32)
        b10 = const.tile([128, 1], f32)

        # per-row statistics, [128, NBLK]-shaped
        s2x = const.tile([128, NBLK], f32)
        rx = const.tile([128, NBLK], f32)
        neg2rx = const.tile([128, NBLK], f32)
        s2y = const.tile([128, YCHUNKS], f32)
        ry = const.tile([128, YCHUNKS], f32)
        gmx = const.tile([128, 2 * NBLK], f32)    # per-half row max of G
        gmax = const.tile([128, NBLK], f32)
        dmin = const.tile([128, NBLK], f32)
        dme = const.tile([128, NBLK], f32)        # dmin + eps
        arec = const.tile([128, NBLK], f32)       # 1/(dmin+eps)
        sc2 = const.tile([128, NBLK], f32)        # -10/(dmin+eps)
        wsh = const.tile([128, 2 * NBLK], f32)    # per-half w row sums
        wsum = const.tile([128, NBLK], f32)
        tdm = const.tile([128, NBLK], f32)        # dmin * arec
        wmax = const.tile([128, NBLK], f32)
        rsum = const.tile([128, NBLK], f32)
        cx = const.tile([128, NBLK], f32)

        # ---------------- load inputs ----------------
        nc.sync.dma_start(out=xw, in_=xw_d)
        nc.sync.dma_start(out=xt, in_=xt_d.rearrange("(k p) c -> p k c", p=128))
        nc.sync.dma_start(out=yt, in_=yt_d.rearrange("(k p) c -> p k c", p=128))

        nc.vector.tensor_copy(xwr, xw)            # rounds f32 -> f32r
        make_identity(nc, ident)
        nc.vector.memset(b2, 2.0)
        nc.vector.memset(b10, 10.0)

        # ---------------- norms (compact layouts) ----------------
        sqx = work.tile([128, NBLK * C], f32, tag="sq")
        nc.vector.tensor_mul(sqx, xt[:, :, :].rearrange("p k c -> p (k c)"),
                             xt[:, :, :].rearrange("p k c -> p (k c)"))
        nc.vector.reduce_sum(s2x, sqx[:].rearrange("p (k c) -> p k c", c=C),
                             axis=AX.X)
        sqy = work.tile([128, YCHUNKS * C], f32, tag="sq")
        nc.vector.tensor_mul(sqy, yt[:, :, :].rearrange("p k c -> p (k c)"),
                             yt[:, :, :].rearrange("p k c -> p (k c)"))
        nc.vector.reduce_sum(s2y, sqy[:].rearrange("p (k c) -> p k c", c=C),
                             axis=AX.X)

        # rx = 1/max(sqrt(s2x), eps); neg2rx = -2*rx
        nrmx = work.tile([128, NBLK], f32, tag="nrm")
        nc.scalar.activation(nrmx, s2x, AF.Sqrt)
        nc.vector.tensor_scalar_max(nrmx, nrmx, EPS_NORM)
        nc.vector.reciprocal(rx, nrmx)
        nc.vector.tensor_scalar_mul(neg2rx, rx, -2.0)

        nrmy = work.tile([128, YCHUNKS], f32, tag="nrm")
        nc.scalar.activation(nrmy, s2y, AF.Sqrt)
        nc.vector.tensor_scalar_max(nrmy, nrmy, EPS_NORM)
        nc.vector.reciprocal(ry, nrmy)

        # ---------------- build yn = y * ry  (via transposed layout) -------
        for k in range(YCHUNKS):
            nc.vector.tensor_scalar_mul(ynt[:, k, :], yt[:, k, :], ry[:, k : k + 1])
        for h in range(2):
            pyn = psum.tile([64, 2048], f32, tag="G")
            for k in range(16):
                kk = 16 * h + k
                nc.tensor.transpose(
                    out=pyn[0:64, 128 * k : 128 * (k + 1)],
                    in_=ynt[:, kk, :],
                    identity=ident,
                )
            # copy PSUM -> SBUF (split across engines)
            if h == 0:
                nc.vector.tensor_copy(yn[0:64, 0:2048], pyn)
            else:
                nc.scalar.copy(yn[0:64, 2048:4096], pyn)

        # Collapse all preproc deps into one barrier: without it the first
        # main-loop LDWEIGHTS accumulates more sync waits than the ISA allows.
        tc.strict_bb_all_engine_barrier()

        # ---------------- main loop ----------------
        # Blocks are processed in groups of GRP so the ACT engine runs all
        # Sqrt passes, then all Exp passes, of a group back-to-back: Sqrt and
        # Exp live in different activation-table sets, and interleaving them
        # costs a ~1.3us ACT_TABLE_LOAD per switch (measured 41us total).
        GRP = 4
        gmxt = const.tile([128, NBLK], f32)  # gmax * (-2rx), batched per group
        last_exp_inst = None
        for g in range(NBLK // GRP):
            dists = {}
            for ii in range(GRP):
                i = g * GRP + ii
                lhsT = xwr[:, 128 * i : 128 * (i + 1)]
                for h in range(2):
                    gt = psum.tile([128, 2048], f32, tag="G")
                    for n in range(4):
                        col = 2048 * h + 512 * n
                        nc.tensor.matmul(
                            gt[:, 512 * n : 512 * (n + 1)],
                            lhsT,
                            yn[0:64, col : col + 512],
                            start=True,
                            stop=True,
                        )
                    ih = 2 * i + h
                    # row max of G (-> min distance)
                    nc.vector.reduce_max(gmx[:, ih : ih + 1], gt, axis=AX.X)
                    # dist = sqrt(2 - 2*rx*G)
                    dist = distp.tile([128, 2048], f32, tag="dist")
                    si = nc.scalar.activation(
                        dist, gt, AF.Sqrt, bias=b2, scale=neg2rx[:, i : i + 1]
                    )
                    if ii == 0 and h == 0 and last_exp_inst is not None:
                        # keep ACT's stream batched: group g's sqrts after
                        # group g-1's exps (scheduler-only ordering)
                        add_dep_helper(si.ins, last_exp_inst, sync=False,
                                       reason="act table batching")
                    dists[(ii, h)] = dist
            # batched per-group scalar chain -> sc2[:, sl]
            sl = slice(g * GRP, (g + 1) * GRP)
            nc.vector.reduce_max(
                gmax[:, sl],
                gmx[:, 2 * g * GRP : 2 * (g + 1) * GRP].rearrange(
                    "p (i h) -> p i h", h=2
                ),
                axis=AX.X,
            )
            nc.vector.tensor_mul(gmxt[:, sl], gmax[:, sl], neg2rx[:, sl])
            nc.scalar.activation(dmin[:, sl], gmxt[:, sl], AF.Sqrt, bias=b2)
            nc.vector.tensor_scalar_add(dme[:, sl], dmin[:, sl], EPS_MIN)
            nc.vector.reciprocal(arec[:, sl], dme[:, sl])
            nc.vector.tensor_scalar_mul(sc2[:, sl], arec[:, sl], -INV_SIGMA)
            for ii in range(GRP):
                i = g * GRP + ii
                for h in range(2):
                    ih = 2 * i + h
                    # w = exp(10 - 10*dist/(dmin+eps)); accumulate row sum
                    wscr = work.tile([128, 2048], f32, tag="wscr")
                    ei = nc.scalar.activation(
                        wscr,
                        dists[(ii, h)],
                        AF.Exp,
                        bias=b10,
                        scale=sc2[:, i : i + 1],
                        accum_out=wsh[:, ih : ih + 1],
                    )
                    last_exp_inst = ei.ins

        # ---------------- epilogue ----------------
        nc.vector.reduce_sum(
            wsum, wsh[:].rearrange("p (i h) -> p i h", h=2), axis=AX.X
        )
        nc.vector.tensor_mul(tdm, dmin, arec)
        nc.scalar.activation(wmax, tdm, AF.Exp, bias=b10, scale=-INV_SIGMA)
        nc.vector.reciprocal(rsum, wsum)
        nc.vector.tensor_mul(cx, wmax, rsum)
        nc.sync.dma_start(out=out_d, in_=cx)

    return nc


def _get_nc(finalized=True):
    """Build (once) and return the Bass module.

    run_bass_via_pjrt serializes the module without calling finalize(), but
    Bacc's legalization passes (register allocation, matmul wait splitting,
    event semaphores) only run in finalize() — so finalize here.
    """
    if "nc" not in _CACHE:
        nc = _build_nc()
        if finalized:
            nc.finalize()
        _CACHE["nc"] = nc
    return _CACHE["nc"]


def _make_in_maps(x, y):
    x = np.ascontiguousarray(np.asarray(x, dtype=np.float32)).reshape(B, C, N)
    y = np.ascontiguousarray(np.asarray(y, dtype=np.float32)).reshape(B, C, N)
    in_maps = []
    for c in range(NCORES):
        b, h = divmod(c, 2)
        xs = x[b][:, 2048 * h : 2048 * (h + 1)]
        in_maps.append(
            {
                "xw": np.ascontiguousarray(xs),
                "xt": np.ascontiguousarray(xs.T),
                "yt": np.ascontiguousarray(y[b].T),
            }
        )
    return in_maps


def _finish(outs):
    """outs: list of 8 arrays [128, NBLK] -> scalar loss (float32 0-d)."""
    total = 0.0
    for o in outs:
        total += float(np.asarray(o, dtype=np.float64).sum())
    mean = total / (B * N) + EPS_MIN
    return np.array(-np.log(mean), dtype=np.float32)


def kernel(x, y):
    from concourse.bass_utils import run_bass_kernel_spmd

    nc = _get_nc()
    in_maps = _make_in_maps(x, y)
    res = run_bass_kernel_spmd(nc, in_maps, core_ids=list(range(NCORES)))
    outs = [res.results[c]["out"] for c in range(NCORES)]
    return _finish(outs)


# revision 10
# speedup vs baseline: 1.4076x; 1.1610x over previous
"""Trainium2 Bass kernel for ContextualLoss.

Reference computation (per batch b):
  xn = x / max(||x||_C, 1e-12)  (channel-wise L2 normalize, C=64)
  yn likewise
  dist[i,j] = || xn[:,i] - yn[:,j] ||  over N=HW=4096 spatial positions
  d_min[i]  = min_j dist[i,j]
  w[i,j]    = exp((1 - dist[i,j]/(d_min[i]+1e-5)) / 0.1)
  cx_max[i] = max_j w[i,j] / sum_j w[i,j]
  loss      = -log(mean(cx_max + 1e-5))

Sharding: 8 cores = 4 batches x 2 row-halves. Each core computes a
[2048 x 4096] block of the distance matrix, fully fused on-chip:
  G = x_raw^T @ yn  (PE, K=C=64)
  dist = Sqrt(-2*rx*G + 2)      (ACT, per-partition scale rx = 1/||x_i||)
  row max of G -> d_min          (DVE reduce)
  w = Exp(-10/(d_min+eps)*dist + 10), row-sum via activation accumulate
  cx_max = w_max / w_sum  -> [128, 16] per core, final -log(mean) on host.
"""

import os
import sys

import numpy as np

sys.path.insert(0, "/opt/trn_rl_repo")

B = 4
C = 64
N = 4096          # H*W
NCORES = 8
ROWS = 2048       # rows of the distance matrix per core (N*B/NCORES)
NBLK = ROWS // 128  # 16 row blocks
YCHUNKS = N // 128  # 32
INV_SIGMA = 10.0    # 1/SIGMA with SIGMA=0.1; B_PARAM=1 -> bias = 10.0
EPS_MIN = 1e-5
EPS_NORM = 1e-12

_CACHE = {}


def _build_nc():
    import concourse.bass as bass
    import concourse.mybir as mybir
    from concourse import bacc, tile
    from concourse.tile import add_dep_helper
    from concourse.masks import make_identity
    from contextlib import ExitStack

    f32 = mybir.dt.float32
    f32r = mybir.dt.float32r
    AF = mybir.ActivationFunctionType
    OP = mybir.AluOpType
    AX = mybir.AxisListType

    nc = bacc.Bacc("TRN2", target_bir_lowering=False, debug=False)

    xw_d = nc.dram_tensor("xw", [C, ROWS], f32, kind="ExternalInput").ap()
    xt_d = nc.dram_tensor("xt", [ROWS, C], f32, kind="ExternalInput").ap()
    yt_d = nc.dram_tensor("yt", [N, C], f32, kind="ExternalInput").ap()
    out_d = nc.dram_tensor("out", [128, NBLK], f32, kind="ExternalOutput").ap()

    with ExitStack() as ctx:
        tc = ctx.enter_context(tile.TileContext(nc))
        const = ctx.enter_context(tc.tile_pool(name="const", bufs=1))
        work = ctx.enter_context(tc.tile_pool(name="work", bufs=3))
        psum = ctx.enter_context(tc.tile_pool(name="psum", bufs=2, space="PSUM"))
        distp = ctx.enter_context(tc.tile_pool(name="distp", bufs=9))

        # ---------------- persistent tiles ----------------
        xw = const.tile([C, ROWS], f32)           # raw x slice (DMA target)
        xwr = const.tile([C, ROWS], f32r)         # f32r-rounded matmul weights
        xt = const.tile([128, NBLK, C], f32)      # x transposed: [p, blk, c]
        yt = const.tile([128, YCHUNKS, C], f32)   # y transposed: [p, chunk, c]
        yn = const.tile([C, N], f32r)             # normalized y, matmul rhs
        ynt = const.tile([128, YCHUNKS, C], f32)  # normalized y, transposed
        ident = const.tile([128, 128], f32)
        b2 = const.tile([128, 1], f32)
        b10 = const.tile([128, 1], f32)

        # per-row statistics, [128, NBLK]-shaped
        s2x = const.tile([128, NBLK], f32)
        rx = const.tile([128, NBLK], f32)
        neg2rx = const.tile([128, NBLK], f32)
        s2y = const.tile([128, YCHUNKS], f32)
        ry = const.tile([128, YCHUNKS], f32)
        gmx = const.tile([128, 2 * NBLK], f32)    # per-half row max of G
        dmin = const.tile([128, NBLK], f32)
        dme = const.tile([128, NBLK], f32)        # dmin + eps
        arec = const.tile([128, NBLK], f32)       # 1/(dmin+eps)
        sc2 = const.tile([128, NBLK], f32)        # -10/(dmin+eps)
        wsh = const.tile([128, 2 * NBLK], f32)    # per-half w row sums
        wsum = const.tile([128, NBLK], f32)
        tdm = const.tile([128, NBLK], f32)        # dmin * arec
        wmax = const.tile([128, NBLK], f32)
        rsum = const.tile([128, NBLK], f32)
        cx = const.tile([128, NBLK], f32)

        # ---------------- load inputs ----------------
        nc.sync.dma_start(out=xw, in_=xw_d)
        nc.sync.dma_start(out=xt, in_=xt_d.rearrange("(k p) c -> p k c", p=128))
        nc.sync.dma_start(out=yt, in_=yt_d.rearrange("(k p) c -> p k c", p=128))

        nc.vector.tensor_copy(xwr, xw)            # rounds f32 -> f32r
        make_identity(nc, ident)
        nc.vector.memset(b2, 2.0)
        nc.vector.memset(b10, 10.0)

        # ---------------- norms (compact layouts) ----------------
        sqx = work.tile([128, NBLK * C], f32, tag="sq")
        nc.vector.tensor_mul(sqx, xt[:, :, :].rearrange("p k c -> p (k c)"),
                             xt[:, :, :].rearrange("p k c -> p (k c)"))
        nc.vector.reduce_sum(s2x, sqx[:].rearrange("p (k c) -> p k c", c=C),
                             axis=AX.X)
        sqy = work.tile([128, YCHUNKS * C], f32, tag="sq")
        nc.vector.tensor_mul(sqy, yt[:, :, :].rearrange("p k c -> p (k c)"),
                             yt[:, :, :].rearrange("p k c -> p (k c)"))
        nc.vector.reduce_sum(s2y, sqy[:].rearrange("p (k c) -> p k c", c=C),
                             axis=AX.X)

        # rx = 1/max(sqrt(s2x), eps); neg2rx = -2*rx
        nrmx = work.tile([128, NBLK], f32, tag="nrm")
        nc.scalar.activation(nrmx, s2x, AF.Sqrt)
        nc.vector.tensor_scalar_max(nrmx, nrmx, EPS_NORM)
        nc.vector.reciprocal(rx, nrmx)
        nc.vector.tensor_scalar_mul(neg2rx, rx, -2.0)

        nrmy = work.tile([128, YCHUNKS], f32, tag="nrm")
        nc.scalar.activation(nrmy, s2y, AF.Sqrt)
        nc.vector.tensor_scalar_max(nrmy, nrmy, EPS_NORM)
        nc.vector.reciprocal(ry, nrmy)

        # ---------------- build yn = y * ry  (via transposed layout) -------
        for k in range(YCHUNKS):
            nc.vector.tensor_scalar_mul(ynt[:, k, :], yt[:, k, :], ry[:, k : k + 1])
        for h in range(2):
            pyn = psum.tile([64, 2048], f32, tag="G")
            for k in range(16):
                kk = 16 * h + k
                nc.tensor.transpose(
                    out=pyn[0:64, 128 * k : 128 * (k + 1)],
                    in_=ynt[:, kk, :],
                    identity=ident,
                )
            # copy PSUM -> SBUF (split across engines)
            if h == 0:
                nc.vector.tensor_copy(yn[0:64, 0:2048], pyn)
            else:
                nc.scalar.copy(yn[0:64, 2048:4096], pyn)

        # ---------------- main loop ----------------
        # Blocks are processed in groups of GRP so the ACT engine runs all
        # Sqrt passes, then all Exp passes, of a group back-to-back: Sqrt and
        # Exp live in different activation-table sets, and interleaving them
        # costs a ~1.3us ACT_TABLE_LOAD per switch (measured 41us total).
        GRP = 4
        last_exp_inst = None
        for g in range(NBLK // GRP):
            dists = {}
            for ii in range(GRP):
                i = g * GRP + ii
                lhsT = xwr[:, 128 * i : 128 * (i + 1)]
                for h in range(2):
                    gt = psum.tile([128, 2048], f32, tag="G")
                    for n in range(4):
                        col = 2048 * h + 512 * n
                        nc.tensor.matmul(
                            gt[:, 512 * n : 512 * (n + 1)],
                            lhsT,
                            yn[0:64, col : col + 512],
                            start=True,
                            stop=True,
                        )
                    ih = 2 * i + h
                    # dist = sqrt(2 - 2*rx*G)
                    dist = distp.tile([128, 2048], f32, tag="dist")
                    si = nc.scalar.activation(
                        dist, gt, AF.Sqrt, bias=b2, scale=neg2rx[:, i : i + 1]
                    )
                    # row min distance (SBUF read; frees the PSUM slot sooner)
                    nc.vector.tensor_reduce(
                        gmx[:, ih : ih + 1], dist, axis=AX.X, op=OP.min
                    )
                    if ii == 0 and h == 0 and last_exp_inst is not None:
                        # keep ACT's stream batched: group g's sqrts after
                        # group g-1's exps (scheduler-only ordering)
                        add_dep_helper(si.ins, last_exp_inst, sync=False,
                                       reason="act table batching")
                    dists[(ii, h)] = dist
            # batched per-group scalar chain -> sc2[:, sl]
            sl = slice(g * GRP, (g + 1) * GRP)
            nc.vector.tensor_reduce(
                dmin[:, sl],
                gmx[:, 2 * g * GRP : 2 * (g + 1) * GRP].rearrange(
                    "p (i h) -> p i h", h=2
                ),
                axis=AX.X,
                op=OP.min,
            )
            nc.vector.tensor_scalar_add(dme[:, sl], dmin[:, sl], EPS_MIN)
            nc.vector.reciprocal(arec[:, sl], dme[:, sl])
            nc.vector.tensor_scalar_mul(sc2[:, sl], arec[:, sl], -INV_SIGMA)
            for ii in range(GRP):
                i = g * GRP + ii
                for h in range(2):
                    ih = 2 * i + h
                    # w = exp(10 - 10*dist/(dmin+eps)); accumulate row sum
                    wscr = work.tile([128, 2048], f32, tag="wscr")
                    ei = nc.scalar.activation(
                        wscr,
                        dists[(ii, h)],
                        AF.Exp,
                        bias=b10,
                        scale=sc2[:, i : i + 1],
                        accum_out=wsh[:, ih : ih + 1],
                    )
                    last_exp_inst = ei.ins

        # ---------------- epilogue ----------------
        nc.vector.reduce_sum(
            wsum, wsh[:].rearrange("p (i h) -> p i h", h=2), axis=AX.X
        )
        nc.vector.tensor_mul(tdm, dmin, arec)
        nc.scalar.activation(wmax, tdm, AF.Exp, bias=b10, scale=-INV_SIGMA)
        nc.vector.reciprocal(rsum, wsum)
        nc.vector.tensor_mul(cx, wmax, rsum)
        nc.sync.dma_start(out=out_d, in_=cx)

    return nc


def _get_nc(finalized=True):
    """Build (once) and return the Bass module.

    run_bass_via_pjrt serializes the module without calling finalize(), but
    Bacc's legalization passes (register allocation, matmul wait splitting,
    event semaphores) only run in finalize() — so finalize here.
    """
    if "nc" not in _CACHE:
        nc = _build_nc()
        if finalized:
            nc.finalize()
        _CACHE["nc"] = nc
    return _CACHE["nc"]


def _make_in_maps(x, y):
    x = np.ascontiguousarray(np.asarray(x, dtype=np.float32)).reshape(B, C, N)
    y = np.ascontiguousarray(np.asarray(y, dtype=np.float32)).reshape(B, C, N)
    in_maps = []
    for c in range(NCORES):
        b, h = divmod(c, 2)
        xs = x[b][:, 2048 * h : 2048 * (h + 1)]
        in_maps.append(
            {
                "xw": np.ascontiguousarray(xs),
                "xt": np.ascontiguousarray(xs.T),
                "yt": np.ascontiguousarray(y[b].T),
            }
        )
    return in_maps


def _finish(outs):
    """outs: list of 8 arrays [128, NBLK] -> scalar loss (float32 0-d)."""
    total = 0.0
    for o in outs:
        total += float(np.asarray(o, dtype=np.float64).sum())
    mean = total / (B * N) + EPS_MIN
    return np.array(-np.log(mean), dtype=np.float32)


def kernel(x, y):
    from concourse.bass_utils import run_bass_kernel_spmd

    nc = _get_nc()
    in_maps = _make_in_maps(x, y)
    res = run_bass_kernel_spmd(nc, in_maps, core_ids=list(range(NCORES)))
    outs = [res.results[c]["out"] for c in range(NCORES)]
    return _finish(outs)


# revision 11
# speedup vs baseline: 1.4808x; 1.0520x over previous
"""Trainium2 Bass kernel for ContextualLoss.

Reference computation (per batch b):
  xn = x / max(||x||_C, 1e-12)  (channel-wise L2 normalize, C=64)
  yn likewise
  dist[i,j] = || xn[:,i] - yn[:,j] ||  over N=HW=4096 spatial positions
  d_min[i]  = min_j dist[i,j]
  w[i,j]    = exp((1 - dist[i,j]/(d_min[i]+1e-5)) / 0.1)
  cx_max[i] = max_j w[i,j] / sum_j w[i,j]
  loss      = -log(mean(cx_max + 1e-5))

Sharding: 8 cores = 4 batches x 2 row-halves. Each core computes a
[2048 x 4096] block of the distance matrix, fully fused on-chip:
  G = x_raw^T @ yn  (PE, K=C=64)
  dist = Sqrt(-2*rx*G + 2)      (ACT, per-partition scale rx = 1/||x_i||)
  row max of G -> d_min          (DVE reduce)
  w = Exp(-10/(d_min+eps)*dist + 10), row-sum via activation accumulate
  cx_max = w_max / w_sum  -> [128, 16] per core, final -log(mean) on host.
"""

import os
import sys

import numpy as np

sys.path.insert(0, "/opt/trn_rl_repo")

B = 4
C = 64
N = 4096          # H*W
NCORES = 8
ROWS = 2048       # rows of the distance matrix per core (N*B/NCORES)
NBLK = ROWS // 128  # 16 row blocks
YCHUNKS = N // 128  # 32
INV_SIGMA = 10.0    # 1/SIGMA with SIGMA=0.1; B_PARAM=1 -> bias = 10.0
EPS_MIN = 1e-5
EPS_NORM = 1e-12

_CACHE = {}


def _build_nc():
    import concourse.bass as bass
    import concourse.mybir as mybir
    from concourse import bacc, tile
    from concourse.tile import add_dep_helper
    from concourse.masks import make_identity
    from contextlib import ExitStack

    f32 = mybir.dt.float32
    f32r = mybir.dt.float32r
    AF = mybir.ActivationFunctionType
    OP = mybir.AluOpType
    AX = mybir.AxisListType

    nc = bacc.Bacc("TRN2", target_bir_lowering=False, debug=False)

    xw_d = nc.dram_tensor("xw", [C, ROWS], f32, kind="ExternalInput").ap()
    xt_d = nc.dram_tensor("xt", [ROWS, C], f32, kind="ExternalInput").ap()
    yt_d = nc.dram_tensor("yt", [N, C], f32, kind="ExternalInput").ap()
    out_d = nc.dram_tensor("out", [128, NBLK], f32, kind="ExternalOutput").ap()

    with ExitStack() as ctx:
        tc = ctx.enter_context(tile.TileContext(nc))
        const = ctx.enter_context(tc.tile_pool(name="const", bufs=1))
        work = ctx.enter_context(tc.tile_pool(name="work", bufs=3))
        psum = ctx.enter_context(tc.tile_pool(name="psum", bufs=2, space="PSUM"))
        distp = ctx.enter_context(tc.tile_pool(name="distp", bufs=17))

        # ---------------- persistent tiles ----------------
        xw = const.tile([C, ROWS], f32)           # raw x slice (DMA target)
        xwr = const.tile([C, ROWS], f32r)         # f32r-rounded matmul weights
        xt = const.tile([128, NBLK, C], f32)      # x transposed: [p, blk, c]
        yt = const.tile([128, YCHUNKS, C], f32)   # y transposed: [p, chunk, c]
        yn = const.tile([C, N], f32r)             # normalized y, matmul rhs
        ynt = const.tile([128, YCHUNKS, C], f32)  # normalized y, transposed
        ident = const.tile([128, 128], f32)
        b2 = const.tile([128, 1], f32)
        b10 = const.tile([128, 1], f32)

        # per-row statistics, [128, NBLK]-shaped
        s2x = const.tile([128, NBLK], f32)
        rx = const.tile([128, NBLK], f32)
        neg2rx = const.tile([128, NBLK], f32)
        s2y = const.tile([128, YCHUNKS], f32)
        ry = const.tile([128, YCHUNKS], f32)
        gmx = const.tile([128, 2 * NBLK], f32)    # per-half row max of G
        dmin = const.tile([128, NBLK], f32)
        dme = const.tile([128, NBLK], f32)        # dmin + eps
        arec = const.tile([128, NBLK], f32)       # 1/(dmin+eps)
        sc2 = const.tile([128, NBLK], f32)        # -10/(dmin+eps)
        wsh = const.tile([128, 2 * NBLK], f32)    # per-half w row sums
        wsum = const.tile([128, NBLK], f32)
        tdm = const.tile([128, NBLK], f32)        # dmin * arec
        wmax = const.tile([128, NBLK], f32)
        rsum = const.tile([128, NBLK], f32)
        cx = const.tile([128, NBLK], f32)

        # ---------------- load inputs ----------------
        nc.sync.dma_start(out=xw, in_=xw_d)
        nc.sync.dma_start(out=xt, in_=xt_d.rearrange("(k p) c -> p k c", p=128))
        yt_v = yt_d.rearrange("(k p) c -> p k c", p=128)
        nc.sync.dma_start(out=yt[:, 0:16, :], in_=yt_v[:, 0:16, :])
        nc.sync.dma_start(out=yt[:, 16:32, :], in_=yt_v[:, 16:32, :])

        nc.vector.tensor_copy(xwr, xw)            # rounds f32 -> f32r
        make_identity(nc, ident)
        nc.vector.memset(b2, 2.0)
        nc.vector.memset(b10, 10.0)

        # ---------------- norms (compact layouts) ----------------
        sqx = work.tile([128, NBLK * C], f32, tag="sq")
        nc.vector.tensor_mul(sqx, xt[:, :, :].rearrange("p k c -> p (k c)"),
                             xt[:, :, :].rearrange("p k c -> p (k c)"))
        nc.vector.reduce_sum(s2x, sqx[:].rearrange("p (k c) -> p k c", c=C),
                             axis=AX.X)

        # rx = 1/max(sqrt(s2x), eps); neg2rx = -2*rx
        nrmx = work.tile([128, NBLK], f32, tag="nrm")
        nc.scalar.activation(nrmx, s2x, AF.Sqrt)
        nc.vector.tensor_scalar_max(nrmx, nrmx, EPS_NORM)
        nc.vector.reciprocal(rx, nrmx)
        nc.vector.tensor_scalar_mul(neg2rx, rx, -2.0)

        # ---------------- y norms + yn, pipelined in column halves -------
        for h in range(2):
            ks = slice(16 * h, 16 * (h + 1))
            sqy = work.tile([128, 16 * C], f32, tag="sq")
            nc.vector.tensor_mul(sqy, yt[:, ks, :].rearrange("p k c -> p (k c)"),
                                 yt[:, ks, :].rearrange("p k c -> p (k c)"))
            nc.vector.reduce_sum(s2y[:, ks], sqy[:].rearrange("p (k c) -> p k c", c=C),
                                 axis=AX.X)
            nrmy = work.tile([128, 16], f32, tag="nrm")
            nc.scalar.activation(nrmy, s2y[:, ks], AF.Sqrt)
            nc.vector.tensor_scalar_max(nrmy, nrmy, EPS_NORM)
            nc.vector.reciprocal(ry[:, ks], nrmy)
            for k in range(16 * h, 16 * (h + 1)):
                nc.vector.tensor_scalar_mul(ynt[:, k, :], yt[:, k, :], ry[:, k : k + 1])
            pyn = psum.tile([64, 2048], f32, tag="G")
            for k in range(16):
                kk = 16 * h + k
                nc.tensor.transpose(
                    out=pyn[0:64, 128 * k : 128 * (k + 1)],
                    in_=ynt[:, kk, :],
                    identity=ident,
                )
            # copy PSUM -> SBUF in 1024-col chunks, alternating engines, so
            # the first matmuls can start before the whole half is copied
            for q in range(2):
                dst = yn[0:64, 2048 * h + 1024 * q : 2048 * h + 1024 * (q + 1)]
                srcp = pyn[0:64, 1024 * q : 1024 * (q + 1)]
                if q == 0:
                    nc.vector.tensor_copy(dst, srcp)
                else:
                    nc.scalar.copy(dst, srcp)

        # ---------------- main loop ----------------
        # Blocks are processed in groups of GRP so the ACT engine runs all
        # Sqrt passes, then all Exp passes, of a group back-to-back: Sqrt and
        # Exp live in different activation-table sets, and interleaving them
        # costs a ~1.3us ACT_TABLE_LOAD per switch (measured 41us total).
        GRP = 8
        last_exp_inst = None
        for g in range(NBLK // GRP):
            dists = {}
            for ii in range(GRP):
                i = g * GRP + ii
                lhsT = xwr[:, 128 * i : 128 * (i + 1)]
                for h in range(2):
                    gt = psum.tile([128, 2048], f32, tag="G")
                    for n in range(4):
                        col = 2048 * h + 512 * n
                        nc.tensor.matmul(
                            gt[:, 512 * n : 512 * (n + 1)],
                            lhsT,
                            yn[0:64, col : col + 512],
                            start=True,
                            stop=True,
                        )
                    ih = 2 * i + h
                    # dist = sqrt(2 - 2*rx*G)
                    dist = distp.tile([128, 2048], mybir.dt.float16, tag="dist")
                    si = nc.scalar.activation(
                        dist, gt, AF.Sqrt, bias=b2, scale=neg2rx[:, i : i + 1]
                    )
                    # row min distance (SBUF read; frees the PSUM slot sooner)
                    nc.vector.tensor_reduce(
                        gmx[:, ih : ih + 1], dist, axis=AX.X, op=OP.min
                    )
                    if ii == 0 and h == 0 and last_exp_inst is not None:
                        # keep ACT's stream batched: group g's sqrts after
                        # group g-1's exps (scheduler-only ordering)
                        add_dep_helper(si.ins, last_exp_inst, sync=False,
                                       reason="act table batching")
                    dists[(ii, h)] = dist
            # batched per-group scalar chain -> sc2[:, sl]
            sl = slice(g * GRP, (g + 1) * GRP)
            nc.vector.tensor_reduce(
                dmin[:, sl],
                gmx[:, 2 * g * GRP : 2 * (g + 1) * GRP].rearrange(
                    "p (i h) -> p i h", h=2
                ),
                axis=AX.X,
                op=OP.min,
            )
            nc.vector.tensor_scalar_add(dme[:, sl], dmin[:, sl], EPS_MIN)
            nc.vector.reciprocal(arec[:, sl], dme[:, sl])
            nc.vector.tensor_scalar_mul(sc2[:, sl], arec[:, sl], -INV_SIGMA)
            for ii in range(GRP):
                i = g * GRP + ii
                for h in range(2):
                    ih = 2 * i + h
                    # w = exp(10 - 10*dist/(dmin+eps)); accumulate row sum
                    wscr = work.tile([128, 2048], f32, tag="wscr")
                    ei = nc.scalar.activation(
                        wscr,
                        dists[(ii, h)],
                        AF.Exp,
                        bias=b10,
                        scale=sc2[:, i : i + 1],
                        accum_out=wsh[:, ih : ih + 1],
                    )
                    last_exp_inst = ei.ins

        # ---------------- epilogue ----------------
        nc.vector.reduce_sum(
            wsum, wsh[:].rearrange("p (i h) -> p i h", h=2), axis=AX.X
        )
        nc.vector.tensor_mul(tdm, dmin, arec)
        nc.scalar.activation(wmax, tdm, AF.Exp, bias=b10, scale=-INV_SIGMA)
        nc.vector.reciprocal(rsum, wsum)
        nc.vector.tensor_mul(cx, wmax, rsum)
        nc.sync.dma_start(out=out_d, in_=cx)

    return nc


def _get_nc(finalized=True):
    """Build (once) and return the Bass module.

    run_bass_via_pjrt serializes the module without calling finalize(), but
    Bacc's legalization passes (register allocation, matmul wait splitting,
    event semaphores) only run in finalize() — so finalize here.
    """
    if "nc" not in _CACHE:
        nc = _build_nc()
        if finalized:
            nc.finalize()
        _CACHE["nc"] = nc
    return _CACHE["nc"]


def _make_in_maps(x, y):
    x = np.ascontiguousarray(np.asarray(x, dtype=np.float32)).reshape(B, C, N)
    y = np.ascontiguousarray(np.asarray(y, dtype=np.float32)).reshape(B, C, N)
    in_maps = []
    for c in range(NCORES):
        b, h = divmod(c, 2)
        xs = x[b][:, 2048 * h : 2048 * (h + 1)]
        in_maps.append(
            {
                "xw": np.ascontiguousarray(xs),
                "xt": np.ascontiguousarray(xs.T),
                "yt": np.ascontiguousarray(y[b].T),
            }
        )
    return in_maps


def _finish(outs):
    """outs: list of 8 arrays [128, NBLK] -> scalar loss (float32 0-d)."""
    total = 0.0
    for o in outs:
        total += float(np.asarray(o, dtype=np.float64).sum())
    mean = total / (B * N) + EPS_MIN
    return np.array(-np.log(mean), dtype=np.float32)


def kernel(x, y):
    from concourse.bass_utils import run_bass_kernel_spmd

    nc = _get_nc()
    in_maps = _make_in_maps(x, y)
    res = run_bass_kernel_spmd(nc, in_maps, core_ids=list(range(NCORES)))
    outs = [res.results[c]["out"] for c in range(NCORES)]
    return _finish(outs)
